# revision 1
# baseline (speedup 1.0000x reference)
"""Trainium2 Bass kernel for nn_Attention_3315714753146 (gnn_message_passing).

out = (LA*softmax(mask(QK^T*scale)) + LG*adj_masked + LD*exp(-dist_masked)) @ V @ W_out + b_out

Sharding: 8 shards = (4 batches) x (2 query-row halves of 512 rows). Each core
computes its own 512 output rows from full K/V (computed on-device from x).
No collectives; gather on host.

Device algorithm per core (bf16 matmul operands, fp32 PSUM accumulation,
fp32 exp paths):
  qT = wq^T @ xT_own, kT = wk^T @ xT, v = xT^T @ wv (head-major weights)
  per head: dotsT[j,i] = kT_h chunks as lhsT against qT_h
            pT = exp(0.125*dotsT + biasj)   (biasj = ln(LA) | -1e30 -> folds LA + col mask)
            pv[65, i] = [v_h | 1/LA]^T @ pT (row 64 = softmax denominator)
            innerT_h = pv[0:64] * (maski/denom)  via PE outer-product broadcast
  C0 = (adj * LG*maski) + LD*maski*exp(-dist)  (fp32); transposed on PE, col-masked
  cvT = V^T @ C0T  (+ outer(LA/N * colsum(V), 1-maski) for invalid query rows)
  out = (innerT + cvT)^T @ W_out + b_out  (bias via K=1 ones outer into PSUM)
"""

import sys

for _p in ("/root/.axon_site", "/root/.axon_site/_ro/trn_rl_repo",
           "/root/.axon_site/_ro/pypackages"):
    if _p not in sys.path:
        sys.path.append(_p)

import numpy as np
import ml_dtypes

BF = ml_dtypes.bfloat16
HEADS, DH = 8, 64
B, N, D = 4, 1024, 512
NH = 512          # query rows per core
LA = LD = LG = 0.33
SCALE = DH ** -0.5
NEG = -1e30
NCORES = 8
# exact compensation for the bf16-rounded 1/LA ones-column in vaug
INV_LA_BF = float(np.float32(BF(1.0 / LA)))

_CACHE = {}


def _build_nc():
    import concourse.bass as bass
    import concourse.bacc as bacc
    import concourse.tile as tile
    from concourse import mybir
    from concourse.bass import ts
    from concourse.masks import make_identity

    F32 = mybir.dt.float32
    BF16 = mybir.dt.bfloat16
    AF = mybir.ActivationFunctionType
    OP = mybir.AluOpType

    nc = bacc.Bacc()
    xT = nc.declare_dram_parameter("xT", [D, N], BF16, isOutput=False)
    xTq = nc.declare_dram_parameter("xTq", [D, NH], BF16, isOutput=False)
    wq = nc.declare_dram_parameter("wq", [D, D], BF16, isOutput=False)
    wk = nc.declare_dram_parameter("wk", [D, D], BF16, isOutput=False)
    wv = nc.declare_dram_parameter("wv", [D, D], BF16, isOutput=False)
    wout = nc.declare_dram_parameter("wout", [D, D], BF16, isOutput=False)
    adj = nc.declare_dram_parameter("adj", [NH, N], F32, isOutput=False)
    dist = nc.declare_dram_parameter("dist", [NH, N], F32, isOutput=False)
    # cvec columns: [0:8]=biasj, [8:16]=maskj, [16:20]=lnldmi, [20:24]=lgmi
    cvec = nc.declare_dram_parameter("cvec", [128, 24], F32, isOutput=False)
    # rvec (bf16): [0:512]=1-maski, [512:1024]=b_out, [1024:1152]=ones,
    #              [1152:1664]=u_row (LA/N * colsum(V), host-computed)
    rvec = nc.declare_dram_parameter("rvec", [1, 1664], BF16, isOutput=False)
    mi8 = nc.declare_dram_parameter("mi8", [8, NH], F32, isOutput=False)
    out = nc.declare_dram_parameter("out", [NH, D], F32, isOutput=True)

    with tile.TileContext(nc) as tc:
        with (
            tc.tile_pool(name="const", bufs=1) as constp,
            tc.tile_pool(name="pers", bufs=1) as pers,
            tc.tile_pool(name="big", bufs=8) as bigp,
            tc.tile_pool(name="med", bufs=8) as medp,
            tc.tile_pool(name="wpool", bufs=8) as wpool,
            tc.tile_pool(name="stage", bufs=2) as stagep,
            tc.tile_pool(name="outp", bufs=2) as outp,
            tc.tile_pool(name="rows", bufs=2) as rowsp,
            tc.tile_pool(name="ppool", bufs=17) as ppool,
            tc.tile_pool(name="pgen", bufs=4, space="PSUM") as pgen,
            tc.tile_pool(name="pacc", bufs=2, space="PSUM") as pacc,
            tc.tile_pool(name="psml", bufs=2, space="PSUM") as psml,
        ):
            ident = constp.tile([128, 128], F32, name="ident")
            make_identity(nc, ident[:])
            onesb = constp.tile([128, 1], BF16, name="onesb")
            nc.gpsimd.memset(onesb[:], 1.0)
            # ---- phase A: projections (one consolidated DMA per tensor,
            # q-path loads first so PE starts early) ----
            def load_w(w_dram, nm):
                t = wpool.tile([128, 4, D], BF16, name=f"{nm}_all", tag="w")
                nc.sync.dma_start(t[:], w_dram[:].rearrange("(c p) d -> p c d", p=128))
                return [t[:, c, :] for c in range(4)]

            wq_sb = load_w(wq, "wq")
            xtq_all = medp.tile([128, 4, NH], BF16, name="xtq_all", tag="medx")
            nc.sync.dma_start(xtq_all[:], xTq[:].rearrange("(c p) d -> p c d", p=128))
            xtq = [xtq_all[:, c, :] for c in range(4)]
            wk_sb = load_w(wk, "wk")
            xt_all = pers.tile([128, 4, N], BF16, name="xt_all")
            nc.sync.dma_start(xt_all[:], xT[:].rearrange("(c p) d -> p c d", p=128))
            xt = [xt_all[:, c, :] for c in range(4)]
            cv = constp.tile([128, 24], F32, name="cv")
            nc.sync.dma_start(cv[:], cvec[:])
            rv = constp.tile([1, 1664], BF16, name="rv")
            nc.sync.dma_start(rv[:], rvec[:])
            omi_row = rv[0:1, 0:NH]
            bout_row = rv[0:1, NH:NH + D]
            ones_row = rv[0:1, 2 * NH:2 * NH + 128]
            u_row = rv[0:1, 2 * NH + 128:2 * NH + 128 + D]

            qt = []
            for oc in range(4):
                ps = pgen.tile([128, NH], F32, name="psw", tag="w")
                for cc in range(4):
                    nc.tensor.matmul(ps[:], lhsT=wq_sb[cc][:, ts(oc, 128)],
                                     rhs=xtq[cc][:], start=(cc == 0), stop=(cc == 3))
                t = pers.tile([128, NH], BF16, name=f"qt{oc}")
                nc.scalar.copy(t[:], ps[:])
                qt.append(t)

            kt = [bigp.tile([128, N], BF16, name=f"kt{oc}", tag="big") for oc in range(4)]
            for oc in range(4):
                for nn in range(2):
                    ps = pgen.tile([128, NH], F32, name="psw", tag="w")
                    for cc in range(4):
                        nc.tensor.matmul(ps[:], lhsT=wk_sb[cc][:, ts(oc, 128)],
                                         rhs=xt[cc][:, ts(nn, 512)],
                                         start=(cc == 0), stop=(cc == 3))
                    nc.scalar.copy(kt[oc][:, ts(nn, 512)], ps[:])

            wv_sb = load_w(wv, "wv")
            vaug = [pers.tile([128, HEADS * 65], BF16, name=f"vaug{j}") for j in range(8)]
            v_pl = [pers.tile([128, D], BF16, name=f"vpl{j}") for j in range(8)]
            for ncc in range(8):
                ps = pgen.tile([128, NH], F32, name="psw", tag="w")
                for cc in range(4):
                    nc.tensor.matmul(ps[:], lhsT=xt[cc][:, ts(ncc, 128)],
                                     rhs=wv_sb[cc][:], start=(cc == 0), stop=(cc == 3))
                nc.vector.tensor_copy(v_pl[ncc][:], ps[:])
                v3 = vaug[ncc][:].rearrange("p (h e) -> p h e", e=65)
                nc.vector.tensor_copy(v3[:, :, 0:64],
                                      v_pl[ncc][:].rearrange("p (h d) -> p h d", d=64))
                nc.gpsimd.memset(v3[:, :, 64:65], 1.0 / LA)

            # ---- phase C0 prep (fp32 exp path) ----
            c0 = [bigp.tile([128, N], F32, name=f"c0{ic}", tag="big") for ic in range(4)]
            for ic in range(4):
                adj_t = stagep.tile([128, N], F32, name="adj_t", tag="adj")
                nc.gpsimd.dma_start(adj_t[:], adj[ts(ic, 128), :])
                dist_t = stagep.tile([128, N], F32, name="dist_t", tag="dist")
                nc.gpsimd.dma_start(dist_t[:], dist[ts(ic, 128), :])
                e_t = stagep.tile([128, N], F32, name="e_t", tag="e")
                nc.scalar.activation(e_t[:], dist_t[:], AF.Exp,
                                     bias=cv[:, 16 + ic:17 + ic], scale=-1.0)
                nc.vector.scalar_tensor_tensor(out=c0[ic][:], in0=adj_t[:],
                                               scalar=cv[:, 20 + ic:21 + ic],
                                               in1=e_t[:], op0=OP.mult, op1=OP.add)

            # ---- phase B1: per-head dots -> exp -> PV; C0 transposes
            # interleaved to fill PE exp-shadow ----
            pv_sb = [medp.tile([65, NH], F32, name=f"pvsb{h}", tag="med") for h in range(HEADS)]
            denoms8 = pers.tile([8, NH], F32, name="denoms8")
            c0T = [pers.tile([128, NH], BF16, name=f"c0T{j}") for j in range(8)]
            for hp in range(4):
                kc = hp
                pts = {0: [], 1: []}
                for jc in range(8):
                    for par in range(2):
                        ko = par * 64
                        dps = pgen.tile([128, NH], F32, name="psw", tag="w")
                        nc.tensor.matmul(dps[:], lhsT=kt[kc][ko:ko + 64, ts(jc, 128)],
                                         rhs=qt[kc][ko:ko + 64, :], start=True,
                                         stop=True, tile_position=(ko, 0))
                        p_t = ppool.tile([128, NH], BF16, name="p_t", tag="p")
                        nc.scalar.activation(p_t[:], dps[:], AF.Exp,
                                             bias=cv[:, jc:jc + 1], scale=SCALE)
                        pts[par].append(p_t)
                for par in range(2):
                    h = 2 * hp + par
                    pvps = pacc.tile([65, NH], F32, name="pvps", tag="a")
                    for jc in range(8):
                        nc.tensor.matmul(pvps[:], lhsT=vaug[jc][:, h * 65:(h + 1) * 65],
                                         rhs=pts[par][jc][:], start=(jc == 0), stop=(jc == 7))
                    nc.vector.tensor_copy(pv_sb[h][:], pvps[:])
                    nc.gpsimd.dma_start(denoms8[h:h + 1, :], pv_sb[h][64:65, :])
                    jc = 2 * hp + par
                    tps = pgen.tile([128, NH], F32, name="psw", tag="w")
                    for ic in range(4):
                        nc.tensor.transpose(tps[:, ts(ic, 128)], c0[ic][:, ts(jc, 128)],
                                            ident[:])
                    nc.vector.tensor_scalar_mul(c0T[jc][:], tps[:], cv[:, 8 + jc:9 + jc])


            # ---- CV accumulation immediately after B1 (fills the reciprocal
            # window); ops groups spread over pacc+psml so all 4 finals pipeline ----
            wout_sb = load_w(wout, "wout")
            cvs = []
            for c2 in range(4):
                cvps = pgen.tile([128, NH], F32, name="cvps", tag="w")
                for jc in range(8):
                    nc.tensor.matmul(cvps[:], lhsT=v_pl[jc][:, ts(c2, 128)],
                                     rhs=c0T[jc][:], start=(jc == 0), stop=False)
                nc.tensor.matmul(cvps[:], lhsT=u_row[0:1, ts(c2, 128)], rhs=omi_row,
                                 start=False, stop=True)
                cvs.append(cvps)

            # ---- phase B2: normalize + mask_i via PE outer broadcast ----
            mi8_sb = constp.tile([8, NH], F32, name="mi8_sb")
            nc.gpsimd.dma_start(mi8_sb[:], mi8[:])
            rec8 = pers.tile([8, NH], F32, name="rec8")
            nc.vector.reciprocal(rec8[:], denoms8[:])
            r8 = pers.tile([8, NH], BF16, name="r8")
            nc.vector.tensor_mul(r8[:], rec8[:], mi8_sb[:])

            ita = [bigp.tile([128, NH], F32, name=f"ita{c}", tag="big") for c in range(4)]
            for h in range(HEADS):
                kc, ko = h // 2, (h % 2) * 64
                r0 = rowsp.tile([1, NH], BF16, name="r0", tag="r0")
                nc.gpsimd.dma_start(r0[:], r8[h:h + 1, :])
                sps = psml.tile([64, NH], F32, name="sps", tag="s")
                nc.tensor.matmul(sps[:], lhsT=ones_row[0:1, 0:64], rhs=r0[:],
                                 start=True, stop=True)
                nc.vector.tensor_mul(ita[kc][ko:ko + 64, :], pv_sb[h][0:64, :], sps[:])

            # ---- final: itf = ita + cv, then out = itf^T @ W_out + b_out ----
            ops = []
            for ic in range(4):
                pool = pacc if ic < 2 else psml
                t = pool.tile([128, D], F32, name=f"ops{ic}", tag="a" if ic < 2 else "s")
                nc.tensor.matmul(t[:], lhsT=ones_row[0:1, 0:128], rhs=bout_row,
                                 start=True, stop=False)
                ops.append(t)
            itf = [bigp.tile([128, NH], BF16, name=f"itf{c}", tag="big") for c in range(4)]
            for c2 in range(4):
                nc.vector.tensor_add(itf[c2][:], ita[c2][:], cvs[c2][:])
                for ic in range(4):
                    nc.tensor.matmul(ops[ic][:], lhsT=itf[c2][:, ts(ic, 128)],
                                     rhs=wout_sb[c2][:], start=False, stop=(c2 == 3))
            for ic in range(4):
                osb = outp.tile([128, D], F32, name="osb", tag="osb")
                nc.vector.tensor_copy(osb[:], ops[ic][:])
                nc.sync.dma_start(out[ts(ic, 128), :], osb[:])

    nc.compile()
    return nc


def get_nc():
    if "nc" not in _CACHE:
        _CACHE["nc"] = _build_nc()
    return _CACHE["nc"]


def make_in_maps(x, mask, adjacency_mat, distance_mat, W_qkv, W_out, b_out):
    x = np.ascontiguousarray(np.asarray(x, np.float32))
    mask = np.asarray(mask)
    adjacency_mat = np.asarray(adjacency_mat, np.float32)
    distance_mat = np.asarray(distance_mat, np.float32)
    W_qkv = np.asarray(W_qkv, np.float32)
    W_out_b = np.ascontiguousarray(np.asarray(W_out, np.float32)).astype(BF)
    b_out = np.asarray(b_out, np.float32)

    W3 = W_qkv.reshape(D, HEADS, 3, DH)
    wq = np.ascontiguousarray(W3[:, :, 0, :].reshape(D, D)).astype(BF)
    wk = np.ascontiguousarray(W3[:, :, 1, :].reshape(D, D)).astype(BF)
    wv = np.ascontiguousarray(W3[:, :, 2, :].reshape(D, D)).astype(BF)

    xT = [np.ascontiguousarray(x[b].T).astype(BF) for b in range(B)]
    wv_f32 = W3[:, :, 2, :].reshape(D, D).astype(np.float32)
    u_host = [(LA / N) * (x[b].sum(0) @ wv_f32) for b in range(B)]

    in_maps = []
    for core in range(NCORES):
        b, half = core // 2, core % 2
        i0 = half * NH
        mj = mask[b].astype(np.float32)
        mi = mask[b, i0:i0 + NH].astype(np.float32)

        biasj = np.where(mj > 0, np.float32(np.log(LA)), np.float32(NEG))
        lnldmi = np.where(mi > 0, np.float32(np.log(LD)), np.float32(NEG))
        lgmi = (LG * mi).astype(np.float32)

        cvec = np.zeros((128, 24), np.float32)
        cvec[:, 0:8] = biasj.reshape(8, 128).T
        cvec[:, 8:16] = mj.reshape(8, 128).T
        cvec[:, 16:20] = lnldmi.reshape(4, 128).T
        cvec[:, 20:24] = lgmi.reshape(4, 128).T

        rvec = np.zeros((1, 1664), np.float32)
        rvec[0, 0:NH] = 1.0 - mi
        rvec[0, NH:NH + D] = b_out
        rvec[0, 2 * NH:2 * NH + 128] = 1.0
        rvec[0, 2 * NH + 128:2 * NH + 128 + D] = u_host[b]

        # mi8 also carries the exact correction for the bf16-rounded 1/LA
        # ones-column: computed softmax part = LA*P/((LA*(1/LA)_bf16)*sumP)
        corr = LA * INV_LA_BF
        mi8v = (np.tile(mi[None, :], (8, 1)) * corr).astype(np.float32)

        in_maps.append({
            "xT": xT[b],
            "xTq": np.ascontiguousarray(x[b, i0:i0 + NH, :].T).astype(BF),
            "wq": wq, "wk": wk, "wv": wv,
            "wout": W_out_b,
            "adj": np.ascontiguousarray(adjacency_mat[b, i0:i0 + NH, :]),
            "dist": np.ascontiguousarray(distance_mat[b, i0:i0 + NH, :]),
            "cvec": cvec,
            "rvec": rvec.astype(BF),
            "mi8": mi8v,
        })
    return in_maps


def kernel(x, mask, adjacency_mat, distance_mat, W_qkv, W_out, b_out):
    from concourse.bass_utils import run_bass_kernel_spmd

    nc = get_nc()
    in_maps = make_in_maps(x, mask, adjacency_mat, distance_mat, W_qkv, W_out, b_out)
    res = run_bass_kernel_spmd(nc, in_maps, core_ids=list(range(NCORES)))
    out_full = np.zeros((B, N, D), np.float32)
    for core in range(NCORES):
        b, half = core // 2, core % 2
        out_full[b, half * NH:(half + 1) * NH, :] = res.results[core]["out"]
    return out_full



# revision 12
# speedup vs baseline: 1.1242x; 1.1242x over previous
"""Trainium2 Bass kernel for nn_Attention_3315714753146 (gnn_message_passing).

out = (LA*softmax(mask(QK^T*scale)) + LG*adj_masked + LD*exp(-dist_masked)) @ V @ W_out + b_out

Sharding: 8 shards = (4 batches) x (2 query-row halves of 512 rows). Each core
computes its own 512 output rows from full K/V (computed on-device from x).
No collectives; gather on host.

Key schedule ideas (PE p-state: full clock only after 3us of continuous run,
so the PE stream is kept dense by software-pipelining):
  - adj/dist are passed host-transposed+masked (adjT=(LG*m2*adj)^T bf16,
    distT=where(m2,dist^T,big)-ln(LD) f32) so c0T = adjT + exp(-distT) needs
    no PE transposes and no on-device mask ops.
  - dots for a (jc, head-pair) go into one 2-bank PSUM tile and are exp'd by
    a single [128,1024] scalar activation (fewer, larger exps).
  - B1 interleaves dots(hp) with v-projection / kt / pv(hp-1) / cv matmuls
    so the PE has independent work while scalar does the exps.
  - B2 normalization via reciprocal_approx_fast on head pairs + one SBUF
    row-gather DMA per pair (no per-head DMA->matmul round trips).
  - out-projection accumulates c2-outer so only the last 4 matmuls depend on
    the final head pair.
"""

import sys

for _p in ("/root/.axon_site", "/root/.axon_site/_ro/trn_rl_repo",
           "/root/.axon_site/_ro/pypackages"):
    if _p not in sys.path:
        sys.path.append(_p)

import numpy as np
import ml_dtypes

BF = ml_dtypes.bfloat16
HEADS, DH = 8, 64
B, N, D = 4, 1024, 512
NH = 512          # query rows per core
LA = LD = LG = 0.33
SCALE = DH ** -0.5
NEG = -1e30
BIG = 1e30
NCORES = 8
# exact compensation for the bf16-rounded 1/LA ones-column in vaug
INV_LA_BF = float(np.float32(BF(1.0 / LA)))

_CACHE = {}


def _build_nc():
    import concourse.bass as bass
    import concourse.bacc as bacc
    import concourse.tile as tile
    from concourse import mybir
    from concourse.bass import ts

    F32 = mybir.dt.float32
    BF16 = mybir.dt.bfloat16
    AF = mybir.ActivationFunctionType
    OP = mybir.AluOpType

    nc = bacc.Bacc()
    xT = nc.declare_dram_parameter("xT", [D, N], BF16, isOutput=False)
    xTq = nc.declare_dram_parameter("xTq", [D, NH], BF16, isOutput=False)
    wq = nc.declare_dram_parameter("wq", [D, D], BF16, isOutput=False)
    wk = nc.declare_dram_parameter("wk", [D, D], BF16, isOutput=False)
    wv = nc.declare_dram_parameter("wv", [D, D], BF16, isOutput=False)
    wout = nc.declare_dram_parameter("wout", [D, D], BF16, isOutput=False)
    adjT = nc.declare_dram_parameter("adjT", [N, NH], BF16, isOutput=False)
    distT = nc.declare_dram_parameter("distT", [N, NH], F32, isOutput=False)
    # cvec columns: [0:8]=biasj (ln(LA) | -1e30 per j, folds LA + col mask)
    cvec = nc.declare_dram_parameter("cvec", [128, 8], F32, isOutput=False)
    # rvec (bf16): [0:512]=1-mi, [512:1024]=b_out, [1024:1152]=ones,
    #              [1152:1664]=u_row (LA/N * colsum(V), host-computed)
    rvec = nc.declare_dram_parameter("rvec", [1, 1664], BF16, isOutput=False)
    mi2 = nc.declare_dram_parameter("mi2", [2, NH], F32, isOutput=False)
    out = nc.declare_dram_parameter("out", [NH, D], F32, isOutput=True)

    with tile.TileContext(nc) as tc:
        with (
            tc.tile_pool(name="const", bufs=1) as constp,
            tc.tile_pool(name="pers", bufs=1) as pers,
            tc.tile_pool(name="wpool", bufs=4) as wpool,
            tc.tile_pool(name="etile", bufs=3) as etp,
            tc.tile_pool(name="outp", bufs=2) as outp,
            tc.tile_pool(name="ppool", bufs=13) as ppool,
            tc.tile_pool(name="pwide", bufs=2, space="PSUM") as pwide,
            tc.tile_pool(name="pgen", bufs=2, space="PSUM") as pgen,
            tc.tile_pool(name="pacc", bufs=2, space="PSUM") as pacc,
        ):
            # ---------------- head DMAs, spread across issue queues ---------
            def load_w(w_dram, nm, eng):
                t = wpool.tile([128, 4, D], BF16, name=f"{nm}_all", tag="w")
                eng.dma_start(t[:], w_dram[:].rearrange("(c p) d -> p c d", p=128))
                return [t[:, c, :] for c in range(4)]

            wq_sb = load_w(wq, "wq", nc.sync)
            xtq_all = pers.tile([128, 4, NH], BF16, name="xtq_all")
            nc.sync.dma_start(xtq_all[:], xTq[:].rearrange("(c p) d -> p c d", p=128))
            xtq = [xtq_all[:, c, :] for c in range(4)]

            wk_sb = load_w(wk, "wk", nc.scalar)
            xt_all = pers.tile([128, 4, N], BF16, name="xt_all")
            nc.scalar.dma_start(xt_all[:], xT[:].rearrange("(c p) d -> p c d", p=128))
            xt = [xt_all[:, c, :] for c in range(4)]

            cv = constp.tile([128, 8], F32, name="cv")
            nc.gpsimd.dma_start(cv[:], cvec[:])
            rv = constp.tile([1, 1664], BF16, name="rv")
            nc.gpsimd.dma_start(rv[:], rvec[:])
            mi2_sb = constp.tile([2, NH], F32, name="mi2_sb")
            nc.gpsimd.dma_start(mi2_sb[:], mi2[:])
            omi_row = rv[0:1, 0:NH]
            bout_row = rv[0:1, NH:NH + D]
            ones_row = rv[0:1, 2 * NH:2 * NH + 128]
            u_row = rv[0:1, 2 * NH + 128:2 * NH + 128 + D]

            wv_sb = load_w(wv, "wv", nc.sync)
            wout_sb = load_w(wout, "wout", nc.sync)

            adjT_sb = pers.tile([128, 8, NH], BF16, name="adjT_sb")
            nc.gpsimd.dma_start(adjT_sb[:], adjT[:].rearrange("(c p) i -> p c i", p=128))
            distT_sb = pers.tile([128, 8, NH], F32, name="distT_sb")
            nc.gpsimd.dma_start(distT_sb[:], distT[:].rearrange("(c p) i -> p c i", p=128))

            # persistent SBUF tensors
            qt = [pers.tile([128, NH], BF16, name=f"qt{oc}") for oc in range(4)]
            kt = [pers.tile([128, N], BF16, name=f"kt{oc}") for oc in range(4)]
            v_pl = [pers.tile([128, D], BF16, name=f"vpl{j}") for j in range(8)]
            vaug = [pers.tile([128, HEADS * 65], BF16, name=f"vaug{j}") for j in range(8)]
            c0T = [pers.tile([128, NH], BF16, name=f"c0T{j}") for j in range(8)]
            pv_sb = [pers.tile([65, NH], F32, name=f"pvsb{h}") for h in range(HEADS)]
            den_p = [pers.tile([2, NH], F32, name=f"denp{k}") for k in range(4)]
            rec_p = [pers.tile([2, NH], F32, name=f"recp{k}") for k in range(4)]
            r8_p = [pers.tile([2, NH], BF16, name=f"r8p{k}") for k in range(4)]
            r0a = pers.tile([1, 8 * NH], BF16, name="r0a")
            ita = [pers.tile([128, NH], BF16, name=f"ita{c}") for c in range(4)]
            cvf = [pers.tile([128, NH], F32, name=f"cvf{c}") for c in range(4)]
            itf = [pers.tile([128, NH], BF16, name=f"itf{c}") for c in range(4)]

            for j in range(8):
                v3 = vaug[j][:].rearrange("p (h e) -> p h e", e=65)
                nc.gpsimd.memset(v3[:, :, 64:65], 1.0 / LA)

            # ---------------- emission helpers ------------------------------
            def qt_group(oc):
                ps = pgen.tile([128, NH], F32, name="psw", tag="w")
                for cc in range(4):
                    nc.tensor.matmul(ps[:], lhsT=wq_sb[cc][:, ts(oc, 128)],
                                     rhs=xtq[cc][:], start=(cc == 0), stop=(cc == 3))
                nc.vector.tensor_copy(qt[oc][:], ps[:])

            def kt_group(oc, nn):
                ps = pgen.tile([128, NH], F32, name="psw", tag="w")
                for cc in range(4):
                    nc.tensor.matmul(ps[:], lhsT=wk_sb[cc][:, ts(oc, 128)],
                                     rhs=xt[cc][:, ts(nn, 512)],
                                     start=(cc == 0), stop=(cc == 3))
                nc.vector.tensor_copy(kt[oc][:, ts(nn, 512)], ps[:])

            def v_group(ncc):
                ps = pgen.tile([128, NH], F32, name="psw", tag="w")
                for cc in range(4):
                    nc.tensor.matmul(ps[:], lhsT=xt[cc][:, ts(ncc, 128)],
                                     rhs=wv_sb[cc][:], start=(cc == 0), stop=(cc == 3))
                nc.vector.tensor_copy(v_pl[ncc][:], ps[:])
                v3 = vaug[ncc][:].rearrange("p (h e) -> p h e", e=65)
                nc.gpsimd.dma_start(v3[:, :, 0:64],
                                    v_pl[ncc][:].rearrange("p (h d) -> p h d", d=64))

            p_tiles = {}  # (hp, jc) -> wide p tile [128, 1024] (par0 | par1)

            def dots_pair(hp, jc):
                kc = hp
                dps = pwide.tile([128, 2 * NH], F32, name="dpsw", tag="dw")
                for par in range(2):
                    ko = par * 64
                    nc.tensor.matmul(dps[:, ts(par, NH)],
                                     lhsT=kt[kc][ko:ko + 64, ts(jc, 128)],
                                     rhs=qt[kc][ko:ko + 64, :], start=True,
                                     stop=True, tile_position=(ko, 0))
                p_t = ppool.tile([128, 2 * NH], BF16, name="p_t", tag="p")
                nc.scalar.activation(p_t[:], dps[:], AF.Exp,
                                     bias=cv[:, jc:jc + 1], scale=SCALE)
                p_tiles[(hp, jc)] = p_t

            pv_ps = {}

            def pv_start(h):
                pv_ps[h] = pacc.tile([65, NH], F32, name="pvps", tag="a")

            def pv_mm(h, jc):
                hp, par = h // 2, h % 2
                nc.tensor.matmul(pv_ps[h][:], lhsT=vaug[jc][:, h * 65:(h + 1) * 65],
                                 rhs=p_tiles[(hp, jc)][:, ts(par, NH)],
                                 start=(jc == 0), stop=(jc == 7))

            def pv_finish(h):
                nc.vector.tensor_copy(pv_sb[h][:], pv_ps[h][:])
                nc.gpsimd.dma_start(den_p[h // 2][h % 2:h % 2 + 1, :],
                                    pv_sb[h][64:65, :])
                del pv_ps[h]

            def c0t_chunk(jc):
                e_t = etp.tile([128, NH], BF16, name="e_t", tag="e")
                nc.scalar.activation(e_t[:], distT_sb[:, jc, :], AF.Exp, scale=-1.0)
                nc.vector.tensor_tensor(out=c0T[jc][:], in0=adjT_sb[:, jc, :],
                                        in1=e_t[:], op=OP.add)

            def b2_heads(h0):
                k = h0 // 2
                nc.vector.reciprocal_approx_fast(rec_p[k][:], den_p[k][:])
                nc.vector.tensor_mul(r8_p[k][:], rec_p[k][:], mi2_sb[:])
                nc.gpsimd.dma_start(r0a[0:1, h0 * NH:(h0 + 2) * NH],
                                    r8_p[k][:])
                for h in (h0, h0 + 1):
                    kc, ko = h // 2, (h % 2) * 64
                    sps = pgen.tile([128, NH], F32, name="sps", tag="w")
                    nc.tensor.matmul(sps[0:64, :], lhsT=ones_row[0:1, 0:64],
                                     rhs=r0a[0:1, h * NH:(h + 1) * NH],
                                     start=True, stop=True)
                    nc.vector.tensor_mul(ita[kc][ko:ko + 64, :], pv_sb[h][0:64, :],
                                         sps[0:64, :])

            def cv_chain(c2):
                cvps = pgen.tile([128, NH], F32, name="cvps", tag="w")
                for jc in range(8):
                    nc.tensor.matmul(cvps[:], lhsT=v_pl[jc][:, ts(c2, 128)],
                                     rhs=c0T[jc][:], start=(jc == 0), stop=False)
                nc.tensor.matmul(cvps[:], lhsT=u_row[0:1, ts(c2, 128)], rhs=omi_row,
                                 start=False, stop=True)
                nc.vector.tensor_copy(cvf[c2][:], cvps[:])

            def itf_add(c2):
                nc.vector.tensor_add(itf[c2][:], ita[c2][:], cvf[c2][:])

            # ---------------- PE program ------------------------------------
            for oc in range(4):
                qt_group(oc)
            kt_group(0, 0)
            kt_group(0, 1)

            # hp=0: dots interleaved with v generation; kt1 at end
            for jc in range(8):
                dots_pair(0, jc)
                v_group(jc)
            kt_group(1, 0)
            kt_group(1, 1)
            for jc in (0, 1, 2):
                c0t_chunk(jc)

            # hp=1: dots(1) interleaved with pv(h0), pv(h1), kt2
            pv_start(0)
            pv_start(1)
            for jc in range(8):
                dots_pair(1, jc)
                pv_mm(0, jc)
                pv_mm(1, jc)
                if jc == 2:
                    kt_group(2, 0)
                elif jc == 5:
                    kt_group(2, 1)
            pv_finish(0)
            pv_finish(1)
            for jc in (3, 4):
                c0t_chunk(jc)

            # hp=2: dots(2) interleaved with pv(h2), pv(h3), kt3, B2(0,1)
            pv_start(2)
            pv_start(3)
            for jc in range(8):
                dots_pair(2, jc)
                pv_mm(2, jc)
                pv_mm(3, jc)
                if jc == 2:
                    kt_group(3, 0)
                elif jc == 4:
                    kt_group(3, 1)
                elif jc == 6:
                    b2_heads(0)
            pv_finish(2)
            pv_finish(3)
            for jc in (5, 6, 7):
                c0t_chunk(jc)

            # hp=3: dots(3) interleaved with pv(h4), pv(h5), cv0/cv1, B2(2,3)
            pv_start(4)
            pv_start(5)
            for jc in range(8):
                dots_pair(3, jc)
                pv_mm(4, jc)
                pv_mm(5, jc)
                if jc == 2:
                    b2_heads(2)
                elif jc == 4:
                    cv_chain(0)
                elif jc == 6:
                    cv_chain(1)
            pv_finish(4)
            pv_finish(5)

            # tail: pv(h6), pv(h7) with cv2/cv3, B2(4,5), out-proj ramp
            pv_start(6)
            pv_start(7)
            for jc in range(8):
                pv_mm(6, jc)
                pv_mm(7, jc)
                if jc == 2:
                    cv_chain(2)
                elif jc == 4:
                    cv_chain(3)
                elif jc == 6:
                    b2_heads(4)
            pv_finish(6)
            pv_finish(7)
            b2_heads(6)

            itf_add(0)
            itf_add(1)
            itf_add(2)
            itf_add(3)

            # out-projection: two wide PSUM tiles hold the 4 row-chunk outputs
            # ops_w[g][:, p*512:...] = bias + sum_c2 itf[c2][ic]^T @ wout[c2]
            ops = []
            for g in range(2):
                t = pwide.tile([128, 2 * NH], F32, name=f"opsw{g}", tag="dw")
                ops.append(t)
            for ic in range(4):
                nc.tensor.matmul(ops[ic // 2][:, ts(ic % 2, NH)],
                                 lhsT=ones_row[0:1, 0:128], rhs=bout_row,
                                 start=True, stop=False)
            for c2 in range(3):
                for ic in range(4):
                    nc.tensor.matmul(ops[ic // 2][:, ts(ic % 2, NH)],
                                     lhsT=itf[c2][:, ts(ic, 128)],
                                     rhs=wout_sb[c2][:], start=False, stop=False)
            for ic in range(4):
                nc.tensor.matmul(ops[ic // 2][:, ts(ic % 2, NH)],
                                 lhsT=itf[3][:, ts(ic, 128)],
                                 rhs=wout_sb[3][:], start=False, stop=True)
                osb = outp.tile([128, NH], F32, name="osb", tag="osb")
                nc.vector.tensor_copy(osb[:], ops[ic // 2][:, ts(ic % 2, NH)])
                nc.sync.dma_start(out[ts(ic, 128), :], osb[:])

    nc.compile()
    return nc


def get_nc():
    if "nc" not in _CACHE:
        _CACHE["nc"] = _build_nc()
    return _CACHE["nc"]


def make_in_maps(x, mask, adjacency_mat, distance_mat, W_qkv, W_out, b_out):
    x = np.ascontiguousarray(np.asarray(x, np.float32))
    mask = np.asarray(mask)
    adjacency_mat = np.asarray(adjacency_mat, np.float32)
    distance_mat = np.asarray(distance_mat, np.float32)
    W_qkv = np.asarray(W_qkv, np.float32)
    W_out_b = np.ascontiguousarray(np.asarray(W_out, np.float32)).astype(BF)
    b_out = np.asarray(b_out, np.float32)

    W3 = W_qkv.reshape(D, HEADS, 3, DH)
    wq = np.ascontiguousarray(W3[:, :, 0, :].reshape(D, D)).astype(BF)
    wk = np.ascontiguousarray(W3[:, :, 1, :].reshape(D, D)).astype(BF)
    wv = np.ascontiguousarray(W3[:, :, 2, :].reshape(D, D)).astype(BF)

    xT = [np.ascontiguousarray(x[b].T).astype(BF) for b in range(B)]
    wv_f32 = W3[:, :, 2, :].reshape(D, D).astype(np.float32)
    u_host = [(LA / N) * (x[b].sum(0) @ wv_f32) for b in range(B)]
    lnld = np.float32(np.log(LD))

    in_maps = []
    for core in range(NCORES):
        b, half = core // 2, core % 2
        i0 = half * NH
        mj = mask[b].astype(np.float32)
        mi = mask[b, i0:i0 + NH].astype(np.float32)
        m2 = mj[:, None] * mi[None, :]  # [N, NH] = (j, i)

        biasj = np.where(mj > 0, np.float32(np.log(LA)), np.float32(NEG))
        cvec = np.ascontiguousarray(biasj.reshape(8, 128).T)

        # adjT = (LG * m2 * adj)^T as bf16 [N, NH]
        adjT = (LG * m2 * adjacency_mat[b, i0:i0 + NH, :].T).astype(BF)
        # distT = where(m2, dist^T, BIG) - ln(LD)  (so exp(-distT) = LD*exp(-d))
        dT = np.where(m2 > 0, distance_mat[b, i0:i0 + NH, :].T, np.float32(BIG))
        distT = np.ascontiguousarray(dT - lnld).astype(np.float32)

        rvec = np.zeros((1, 1664), np.float32)
        rvec[0, 0:NH] = 1.0 - mi
        rvec[0, NH:NH + D] = b_out
        rvec[0, 2 * NH:2 * NH + 128] = 1.0
        rvec[0, 2 * NH + 128:2 * NH + 128 + D] = u_host[b]

        # mi2 also carries the exact correction for the bf16-rounded 1/LA
        # ones-column: computed softmax part = LA*P/((LA*(1/LA)_bf16)*sumP)
        corr = LA * INV_LA_BF
        mi2v = (np.tile(mi[None, :], (2, 1)) * corr).astype(np.float32)

        in_maps.append({
            "xT": xT[b],
            "xTq": np.ascontiguousarray(x[b, i0:i0 + NH, :].T).astype(BF),
            "wq": wq, "wk": wk, "wv": wv,
            "wout": W_out_b,
            "adjT": np.ascontiguousarray(adjT),
            "distT": distT,
            "cvec": cvec,
            "rvec": rvec.astype(BF),
            "mi2": mi2v,
        })
    return in_maps


def kernel(x, mask, adjacency_mat, distance_mat, W_qkv, W_out, b_out):
    from concourse.bass_utils import run_bass_kernel_spmd

    nc = get_nc()
    in_maps = make_in_maps(x, mask, adjacency_mat, distance_mat, W_qkv, W_out, b_out)
    res = run_bass_kernel_spmd(nc, in_maps, core_ids=list(range(NCORES)))
    out_full = np.zeros((B, N, D), np.float32)
    for core in range(NCORES):
        b, half = core // 2, core % 2
        out_full[b, half * NH:(half + 1) * NH, :] = res.results[core]["out"]
    return out_full


# revision 21
# speedup vs baseline: 1.1257x; 1.0013x over previous
"""Trainium2 Bass kernel for nn_Attention_3315714753146 (gnn_message_passing).

out = (LA*softmax(mask(QK^T*scale)) + LG*adj_masked + LD*exp(-dist_masked)) @ V @ W_out + b_out

Sharding: 8 shards = (4 batches) x (2 query-row halves of 512 rows). Each core
computes its own 512 output rows from full K/V (computed on-device from x).
No collectives; gather on host.

Key schedule ideas (PE p-state: full clock only after 3us of continuous run,
so the PE stream is kept dense by software-pipelining):
  - adj/dist are passed host-transposed+masked (adjT=(LG*m2*adj)^T bf16,
    distT=where(m2,dist^T,big)-ln(LD) f32) so c0T = adjT + exp(-distT) needs
    no PE transposes and no on-device mask ops.
  - dots for a (jc, head-pair) go into one 2-bank PSUM tile and are exp'd by
    a single [128,1024] scalar activation (fewer, larger exps).
  - B1 interleaves dots(hp) with v-projection / kt / pv(hp-1) / cv matmuls
    so the PE has independent work while scalar does the exps.
  - B2 normalization via reciprocal_approx_fast on head pairs + one SBUF
    row-gather DMA per pair (no per-head DMA->matmul round trips).
  - out-projection accumulates c2-outer so only the last 4 matmuls depend on
    the final head pair.
"""

import sys

for _p in ("/root/.axon_site", "/root/.axon_site/_ro/trn_rl_repo",
           "/root/.axon_site/_ro/pypackages"):
    if _p not in sys.path:
        sys.path.append(_p)

import numpy as np
import ml_dtypes

BF = ml_dtypes.bfloat16
HEADS, DH = 8, 64
B, N, D = 4, 1024, 512
NH = 512          # query rows per core
LA = LD = LG = 0.33
SCALE = DH ** -0.5
NEG = -1e30
BIG = 1e30
NCORES = 8
# exact compensation for the bf16-rounded 1/LA ones-column in vaug
INV_LA_BF = float(np.float32(BF(1.0 / LA)))

_CACHE = {}


def _build_nc():
    import concourse.bass as bass
    import concourse.bacc as bacc
    import concourse.tile as tile
    from concourse import mybir
    from concourse.bass import ts

    F32 = mybir.dt.float32
    BF16 = mybir.dt.bfloat16
    AF = mybir.ActivationFunctionType
    OP = mybir.AluOpType

    nc = bacc.Bacc()
    xT = nc.declare_dram_parameter("xT", [D, N], BF16, isOutput=False)
    xTq = nc.declare_dram_parameter("xTq", [D, NH], BF16, isOutput=False)
    wq = nc.declare_dram_parameter("wq", [D, D], BF16, isOutput=False)
    wk = nc.declare_dram_parameter("wk", [D, D], BF16, isOutput=False)
    wv = nc.declare_dram_parameter("wv", [D, D], BF16, isOutput=False)
    wout = nc.declare_dram_parameter("wout", [D, D], BF16, isOutput=False)
    adjT = nc.declare_dram_parameter("adjT", [N, NH], BF16, isOutput=False)
    distT = nc.declare_dram_parameter("distT", [N, NH], F32, isOutput=False)
    # cvec columns: [0:8]=biasj (ln(LA) | -1e30 per j, folds LA + col mask)
    cvec = nc.declare_dram_parameter("cvec", [128, 8], F32, isOutput=False)
    # rvec (bf16): [0:512]=1-mi, [512:1024]=b_out, [1024:1152]=ones,
    #              [1152:1664]=u_row (LA/N * colsum(V), host-computed)
    rvec = nc.declare_dram_parameter("rvec", [1, 1664], BF16, isOutput=False)
    mi2 = nc.declare_dram_parameter("mi2", [2, NH], F32, isOutput=False)
    out = nc.declare_dram_parameter("out", [NH, D], F32, isOutput=True)

    with tile.TileContext(nc) as tc:
        with (
            tc.tile_pool(name="const", bufs=1) as constp,
            tc.tile_pool(name="pers", bufs=1) as pers,
            tc.tile_pool(name="wpool", bufs=4) as wpool,
            tc.tile_pool(name="etile", bufs=3) as etp,
            tc.tile_pool(name="outp", bufs=2) as outp,
            tc.tile_pool(name="ppool", bufs=13) as ppool,
            tc.tile_pool(name="pwide", bufs=2, space="PSUM") as pwide,
            tc.tile_pool(name="pgen", bufs=2, space="PSUM") as pgen,
            tc.tile_pool(name="pacc", bufs=2, space="PSUM") as pacc,
        ):
            # ---------------- head DMAs, spread across issue queues ---------
            def load_w(w_dram, nm, eng):
                t = wpool.tile([128, 4, D], BF16, name=f"{nm}_all", tag="w")
                eng.dma_start(t[:], w_dram[:].rearrange("(c p) d -> p c d", p=128))
                return [t[:, c, :] for c in range(4)]

            wq_sb = load_w(wq, "wq", nc.sync)
            xtq_all = pers.tile([128, 4, NH], BF16, name="xtq_all")
            nc.sync.dma_start(xtq_all[:], xTq[:].rearrange("(c p) d -> p c d", p=128))
            xtq = [xtq_all[:, c, :] for c in range(4)]

            wk_sb = load_w(wk, "wk", nc.scalar)
            # xt in two n-halves so kt[0] can start before the full 1MB lands
            xt_all = pers.tile([128, 4, N], BF16, name="xt_all")
            for nh_ in range(2):
                nc.sync.dma_start(
                    xt_all[:, :, ts(nh_, 512)],
                    xT[:, ts(nh_, 512)].rearrange("(c p) d -> p c d", p=128))
            xt = [xt_all[:, c, :] for c in range(4)]

            cv = constp.tile([128, 8], F32, name="cv")
            nc.gpsimd.dma_start(cv[:], cvec[:])
            rv = constp.tile([1, 1664], BF16, name="rv")
            nc.gpsimd.dma_start(rv[:], rvec[:])
            mi2_sb = constp.tile([2, NH], F32, name="mi2_sb")
            nc.gpsimd.dma_start(mi2_sb[:], mi2[:])
            omi_row = rv[0:1, 0:NH]
            bout_row = rv[0:1, NH:NH + D]
            ones_row = rv[0:1, 2 * NH:2 * NH + 128]
            u_row = rv[0:1, 2 * NH + 128:2 * NH + 128 + D]

            wv_sb = load_w(wv, "wv", nc.sync)
            # wout + adjT/distT are issued later (on gpsimd, behind v_group
            # progress) so the early critical DMAs get full HBM bandwidth
            wout_all = wpool.tile([128, 4, D], BF16, name="wout_all", tag="w")
            wout_sb = [wout_all[:, c, :] for c in range(4)]

            adjp = [pers.tile([128, 2, NH], BF16, name=f"adjp{k}") for k in range(4)]
            distp = [pers.tile([128, 2, NH], F32, name=f"distp{k}") for k in range(4)]

            def adj_dist_pair(k):
                nc.gpsimd.dma_start(
                    adjp[k][:],
                    adjT[2 * k * 128:(2 * k + 2) * 128, :]
                    .rearrange("(c p) i -> p c i", p=128))
                nc.gpsimd.dma_start(
                    distp[k][:],
                    distT[2 * k * 128:(2 * k + 2) * 128, :]
                    .rearrange("(c p) i -> p c i", p=128))

            # persistent SBUF tensors
            qt = [pers.tile([128, NH], BF16, name=f"qt{oc}") for oc in range(4)]
            kt = [pers.tile([128, N], BF16, name=f"kt{oc}") for oc in range(4)]
            v_pl = [pers.tile([128, D], BF16, name=f"vpl{j}") for j in range(8)]
            vaug = [pers.tile([128, HEADS * 65], BF16, name=f"vaug{j}") for j in range(8)]
            c0T = [pers.tile([128, NH], BF16, name=f"c0T{j}") for j in range(8)]
            pv_sb = [pers.tile([65, NH], F32, name=f"pvsb{h}") for h in range(HEADS)]
            # pair tiles for heads 0..5; single-row tiles for 6,7 (so the two
            # tail chains run independently and start at partition 0)
            den_p = [pers.tile([2, NH], F32, name=f"denp{k}") for k in range(3)]
            rec_p = [pers.tile([2, NH], F32, name=f"recp{k}") for k in range(3)]
            r8_p = [pers.tile([2, NH], BF16, name=f"r8p{k}") for k in range(3)]
            den_s = {h: pers.tile([1, NH], F32, name=f"dens{h}") for h in (6, 7)}
            rec_s = {h: pers.tile([1, NH], F32, name=f"recs{h}") for h in (6, 7)}
            r8_s = {h: pers.tile([1, NH], BF16, name=f"r8s{h}") for h in (6, 7)}
            r0a = pers.tile([1, 8 * NH], BF16, name="r0a")
            ita = [pers.tile([128, NH], BF16, name=f"ita{c}") for c in range(4)]
            cvf = [pers.tile([128, NH], F32, name=f"cvf{c}") for c in range(4)]
            itf = [pers.tile([128, NH], BF16, name=f"itf{c}") for c in range(4)]

            for j in range(8):
                v3 = vaug[j][:].rearrange("p (h e) -> p h e", e=65)
                nc.gpsimd.memset(v3[:, :, 64:65], 1.0 / LA)

            # ---------------- emission helpers ------------------------------
            def qt_group(oc):
                ps = pgen.tile([128, NH], F32, name="psw", tag="w")
                for cc in range(4):
                    nc.tensor.matmul(ps[:], lhsT=wq_sb[cc][:, ts(oc, 128)],
                                     rhs=xtq[cc][:], start=(cc == 0), stop=(cc == 3))
                nc.vector.tensor_copy(qt[oc][:], ps[:])

            def kt_group(oc, nn):
                ps = pgen.tile([128, NH], F32, name="psw", tag="w")
                for cc in range(4):
                    nc.tensor.matmul(ps[:], lhsT=wk_sb[cc][:, ts(oc, 128)],
                                     rhs=xt[cc][:, ts(nn, 512)],
                                     start=(cc == 0), stop=(cc == 3))
                nc.vector.tensor_copy(kt[oc][:, ts(nn, 512)], ps[:])

            def v_group(ncc):
                ps = pgen.tile([128, NH], F32, name="psw", tag="w")
                for cc in range(4):
                    nc.tensor.matmul(ps[:], lhsT=xt[cc][:, ts(ncc, 128)],
                                     rhs=wv_sb[cc][:], start=(cc == 0), stop=(cc == 3))
                nc.vector.tensor_copy(v_pl[ncc][:], ps[:])
                v3 = vaug[ncc][:].rearrange("p (h e) -> p h e", e=65)
                nc.gpsimd.dma_start(v3[:, :, 0:64],
                                    v_pl[ncc][:].rearrange("p (h d) -> p h d", d=64))

            p_tiles = {}  # (hp, jc) -> wide p tile [128, 1024] (par0 | par1)

            def dots_pair(hp, jc):
                kc = hp
                dps = pwide.tile([128, 2 * NH], F32, name="dpsw", tag="dw")
                for par in range(2):
                    ko = par * 64
                    nc.tensor.matmul(dps[:, ts(par, NH)],
                                     lhsT=kt[kc][ko:ko + 64, ts(jc, 128)],
                                     rhs=qt[kc][ko:ko + 64, :], start=True,
                                     stop=True, tile_position=(ko, 0))
                p_t = ppool.tile([128, 2 * NH], BF16, name="p_t", tag="p")
                nc.scalar.activation(p_t[:], dps[:], AF.Exp,
                                     bias=cv[:, jc:jc + 1], scale=SCALE)
                p_tiles[(hp, jc)] = p_t

            pv_ps = {}

            def pv_start(h):
                pv_ps[h] = pacc.tile([65, NH], F32, name="pvps", tag="a")

            def pv_mm(h, jc):
                hp, par = h // 2, h % 2
                nc.tensor.matmul(pv_ps[h][:], lhsT=vaug[jc][:, h * 65:(h + 1) * 65],
                                 rhs=p_tiles[(hp, jc)][:, ts(par, NH)],
                                 start=(jc == 0), stop=(jc == 7))

            def pv_finish(h):
                nc.vector.tensor_copy(pv_sb[h][:], pv_ps[h][:])
                if h >= 6:
                    nc.gpsimd.dma_start(den_s[h][:], pv_sb[h][64:65, :])
                else:
                    nc.gpsimd.dma_start(den_p[h // 2][h % 2:h % 2 + 1, :],
                                        pv_sb[h][64:65, :])
                del pv_ps[h]

            def c0t_chunk(jc):
                e_t = etp.tile([128, NH], BF16, name="e_t", tag="e")
                nc.scalar.activation(e_t[:], distp[jc // 2][:, jc % 2, :],
                                     AF.Exp, scale=-1.0)
                nc.vector.tensor_tensor(out=c0T[jc][:], in0=adjp[jc // 2][:, jc % 2, :],
                                        in1=e_t[:], op=OP.add)

            def b2_sps(h):
                kc, ko = h // 2, (h % 2) * 64
                sps = pgen.tile([128, NH], F32, name="sps", tag="w")
                nc.tensor.matmul(sps[0:64, :], lhsT=ones_row[0:1, 0:64],
                                 rhs=r0a[0:1, h * NH:(h + 1) * NH],
                                 start=True, stop=True)
                nc.vector.tensor_mul(ita[kc][ko:ko + 64, :], pv_sb[h][0:64, :],
                                     sps[0:64, :])

            def b2_heads(h0):
                k = h0 // 2
                nc.vector.reciprocal_approx_fast(rec_p[k][:], den_p[k][:])
                nc.vector.tensor_mul(r8_p[k][:], rec_p[k][:], mi2_sb[:])
                nc.gpsimd.dma_start(r0a[0:1, h0 * NH:(h0 + 2) * NH],
                                    r8_p[k][:])
                b2_sps(h0)
                b2_sps(h0 + 1)

            def b2_single_pre(h):
                nc.vector.reciprocal_approx_fast(rec_s[h][:], den_s[h][:])
                nc.vector.tensor_mul(r8_s[h][:], rec_s[h][:], mi2_sb[0:1, :])
                nc.gpsimd.dma_start(r0a[0:1, h * NH:(h + 1) * NH], r8_s[h][:])

            def cv_chain(c2):
                cvps = pgen.tile([128, NH], F32, name="cvps", tag="w")
                for jc in range(8):
                    nc.tensor.matmul(cvps[:], lhsT=v_pl[jc][:, ts(c2, 128)],
                                     rhs=c0T[jc][:], start=(jc == 0), stop=False)
                nc.tensor.matmul(cvps[:], lhsT=u_row[0:1, ts(c2, 128)], rhs=omi_row,
                                 start=False, stop=True)
                nc.vector.tensor_copy(cvf[c2][:], cvps[:])

            def itf_add(c2):
                nc.vector.tensor_add(itf[c2][:], ita[c2][:], cvf[c2][:])

            # ---------------- PE program ------------------------------------
            for oc in range(4):
                qt_group(oc)
            kt_group(0, 0)
            kt_group(0, 1)

            # hp=0: dots interleaved with v generation; kt1 at end.
            # adjT/distT chunk DMAs trail v_group progress on the gpsimd queue
            # so early critical DMAs get full HBM bandwidth; wout last.
            for jc in range(8):
                dots_pair(0, jc)
                v_group(jc)
                if jc < 4:
                    adj_dist_pair(jc)
                elif jc == 4:
                    nc.gpsimd.dma_start(
                        wout_all[:], wout[:].rearrange("(c p) d -> p c d", p=128))
            kt_group(1, 0)
            kt_group(1, 1)
            for jc in (0, 1, 2):
                c0t_chunk(jc)

            # hp=1: dots(1) interleaved with pv(h0), pv(h1), kt2
            pv_start(0)
            pv_start(1)
            for jc in range(8):
                dots_pair(1, jc)
                pv_mm(0, jc)
                pv_mm(1, jc)
                if jc == 2:
                    kt_group(2, 0)
                elif jc == 5:
                    kt_group(2, 1)
            pv_finish(0)
            pv_finish(1)
            for jc in (3, 4):
                c0t_chunk(jc)

            # hp=2: dots(2) interleaved with pv(h2), pv(h3), kt3, B2(0,1)
            pv_start(2)
            pv_start(3)
            for jc in range(8):
                dots_pair(2, jc)
                pv_mm(2, jc)
                pv_mm(3, jc)
                if jc == 2:
                    kt_group(3, 0)
                elif jc == 4:
                    kt_group(3, 1)
                elif jc == 6:
                    b2_heads(0)
            pv_finish(2)
            pv_finish(3)
            for jc in (5, 6, 7):
                c0t_chunk(jc)

            # hp=3: dots(3) interleaved with pv(h4), pv(h5), cv0/cv1, B2(2,3)
            pv_start(4)
            pv_start(5)
            for jc in range(8):
                dots_pair(3, jc)
                pv_mm(4, jc)
                pv_mm(5, jc)
                if jc == 2:
                    b2_heads(2)
                elif jc == 4:
                    cv_chain(0)
                elif jc == 6:
                    cv_chain(1)
            pv_finish(4)
            pv_finish(5)

            # tail: pv(h6), pv(h7) with cv2/cv3, B2(4,5), out-proj ramp
            pv_start(6)
            pv_start(7)
            for jc in range(8):
                pv_mm(6, jc)
                pv_mm(7, jc)
                if jc == 2:
                    cv_chain(2)
                elif jc == 4:
                    cv_chain(3)
                elif jc == 6:
                    b2_heads(4)
            pv_finish(6)
            b2_single_pre(6)
            pv_finish(7)
            b2_single_pre(7)
            itf_add(0)
            itf_add(1)
            itf_add(2)

            # out-projection filler (bias + c2<=2) runs on PE while the h6/h7
            # normalization chains flow through vector/gpsimd
            ops = []
            for g in range(2):
                t = pwide.tile([128, 2 * NH], F32, name=f"opsw{g}", tag="dw")
                ops.append(t)
            for ic in range(4):
                nc.tensor.matmul(ops[ic // 2][:, ts(ic % 2, NH)],
                                 lhsT=ones_row[0:1, 0:128], rhs=bout_row,
                                 start=True, stop=False)
            b2_sps(6)
            for c2 in range(3):
                for ic in range(4):
                    nc.tensor.matmul(ops[ic // 2][:, ts(ic % 2, NH)],
                                     lhsT=itf[c2][:, ts(ic, 128)],
                                     rhs=wout_sb[c2][:], start=False, stop=False)
            b2_sps(7)
            itf_add(3)
            for ic in range(4):
                nc.tensor.matmul(ops[ic // 2][:, ts(ic % 2, NH)],
                                 lhsT=itf[3][:, ts(ic, 128)],
                                 rhs=wout_sb[3][:], start=False, stop=True)
                osb = outp.tile([128, NH], F32, name="osb", tag="osb")
                if ic % 2 == 0:
                    nc.vector.tensor_copy(osb[:], ops[ic // 2][:, ts(ic % 2, NH)])
                else:
                    nc.scalar.copy(osb[:], ops[ic // 2][:, ts(ic % 2, NH)])
                nc.sync.dma_start(out[ts(ic, 128), :], osb[:])

    nc.compile()
    return nc


def get_nc():
    if "nc" not in _CACHE:
        _CACHE["nc"] = _build_nc()
    return _CACHE["nc"]


def make_in_maps(x, mask, adjacency_mat, distance_mat, W_qkv, W_out, b_out):
    x = np.ascontiguousarray(np.asarray(x, np.float32))
    mask = np.asarray(mask)
    adjacency_mat = np.asarray(adjacency_mat, np.float32)
    distance_mat = np.asarray(distance_mat, np.float32)
    W_qkv = np.asarray(W_qkv, np.float32)
    W_out_b = np.ascontiguousarray(np.asarray(W_out, np.float32)).astype(BF)
    b_out = np.asarray(b_out, np.float32)

    W3 = W_qkv.reshape(D, HEADS, 3, DH)
    wq = np.ascontiguousarray(W3[:, :, 0, :].reshape(D, D)).astype(BF)
    wk = np.ascontiguousarray(W3[:, :, 1, :].reshape(D, D)).astype(BF)
    wv = np.ascontiguousarray(W3[:, :, 2, :].reshape(D, D)).astype(BF)

    xT = [np.ascontiguousarray(x[b].T).astype(BF) for b in range(B)]
    wv_f32 = W3[:, :, 2, :].reshape(D, D).astype(np.float32)
    u_host = [(LA / N) * (x[b].sum(0) @ wv_f32) for b in range(B)]
    lnld = np.float32(np.log(LD))

    in_maps = []
    for core in range(NCORES):
        b, half = core // 2, core % 2
        i0 = half * NH
        mj = mask[b].astype(np.float32)
        mi = mask[b, i0:i0 + NH].astype(np.float32)
        m2 = mj[:, None] * mi[None, :]  # [N, NH] = (j, i)

        biasj = np.where(mj > 0, np.float32(np.log(LA)), np.float32(NEG))
        cvec = np.ascontiguousarray(biasj.reshape(8, 128).T)

        # adjT = (LG * m2 * adj)^T as bf16 [N, NH]
        adjT = (LG * m2 * adjacency_mat[b, i0:i0 + NH, :].T).astype(BF)
        # distT = where(m2, dist^T, BIG) - ln(LD)  (so exp(-distT) = LD*exp(-d))
        dT = np.where(m2 > 0, distance_mat[b, i0:i0 + NH, :].T, np.float32(BIG))
        distT = np.ascontiguousarray(dT - lnld).astype(np.float32)

        rvec = np.zeros((1, 1664), np.float32)
        rvec[0, 0:NH] = 1.0 - mi
        rvec[0, NH:NH + D] = b_out
        rvec[0, 2 * NH:2 * NH + 128] = 1.0
        rvec[0, 2 * NH + 128:2 * NH + 128 + D] = u_host[b]

        # mi2 also carries the exact correction for the bf16-rounded 1/LA
        # ones-column: computed softmax part = LA*P/((LA*(1/LA)_bf16)*sumP)
        corr = LA * INV_LA_BF
        mi2v = (np.tile(mi[None, :], (2, 1)) * corr).astype(np.float32)

        in_maps.append({
            "xT": xT[b],
            "xTq": np.ascontiguousarray(x[b, i0:i0 + NH, :].T).astype(BF),
            "wq": wq, "wk": wk, "wv": wv,
            "wout": W_out_b,
            "adjT": np.ascontiguousarray(adjT),
            "distT": distT,
            "cvec": cvec,
            "rvec": rvec.astype(BF),
            "mi2": mi2v,
        })
    return in_maps


def kernel(x, mask, adjacency_mat, distance_mat, W_qkv, W_out, b_out):
    from concourse.bass_utils import run_bass_kernel_spmd

    nc = get_nc()
    in_maps = make_in_maps(x, mask, adjacency_mat, distance_mat, W_qkv, W_out, b_out)
    res = run_bass_kernel_spmd(nc, in_maps, core_ids=list(range(NCORES)))
    out_full = np.zeros((B, N, D), np.float32)
    for core in range(NCORES):
        b, half = core // 2, core % 2
        out_full[b, half * NH:(half + 1) * NH, :] = res.results[core]["out"]
    return out_full
